# revision 14
# baseline (speedup 1.0000x reference)
"""CondMlp Trainium2 kernel (v3).

Math (reference):
    xp = x @ W_pre + b_pre                 # [B, NI, DH]
    c  = query @ W_emb + b_emb             # [B, NO, DH]
    A  = xp @ W1[:DH] + b1                 # [B, NI, DH]   (host precompute, tiny)
    C2 = c @ W1[DH:]                       # [B, NO, DH]   (host precompute, tiny)
    h[b,i,o,:] = A[b,i,:] + C2[b,o,:]
    out[b,i,o,:] = gelu(h) @ W2 + b2       # [B, NI, NO, DOUT]

Sharding: 8 cores, core k handles batch b = k//2, NI-half h = k%2 (128 rows).

Design (measured constants from traces/microbench):
  * The kernel is ACT+DVE-bound: per core the gelu (58us, ACT-only), the
    broadcast adds (50us, DVE tensor_scalar at 2x_1P, 197ns/[128,256]) and
    the PSUM drains (1x port-bound; ACT 1.97us / DVE 2.29us per [128,2048])
    must share two engines -> ~89us/engine balanced.
  * bf16 output stores (halves DMA) assembled/untransposed on host.
  * W2-stationary matmuls, N=512: back-to-back same-lhsT MMs run at stream
    rate (216ns, LDWEIGHTS hidden by the PE background weight buffer).
  * Drains split ACT/DVE ~15/17 (the LP balance point).
  * 8-row first/last groups + per-subgroup 512KB stores shorten the
    pipeline head/tail; PE warmup matmuls flip the HAM clock-gate early.
  * GPSIMD stock tensor_scalar measured 3.9us per [128,256] add (20x DVE):
    useless for compute; its SWDGE ring only carries the w2 loads.
"""

import numpy as np
import ml_dtypes

import concourse.bass as bass
import concourse.bacc as bacc
import concourse.mybir as mybir
from concourse.tile import TileContext
from concourse.bass_utils import run_bass_kernel_spmd

B, NI, NO = 4, 256, 256
DIN, DQ, DH, DOUT = 256, 256, 256, 256
NCORES = 8
RPC = (B * NI) // NCORES    # rows per core = 128
NSUB = RPC // 4             # 32 matmul subgroups of 4 rows
GROUP_ROWS = [8, 16, 32, 32, 32, 8]   # taper head/tail, big middle gelus
F32 = mybir.dt.float32
BF16 = mybir.dt.bfloat16

ACT_DRAINS = 15             # of 32 drains, how many go to ACT (rest DVE)

_nc_cache = None


def build_nc():
    nc = bacc.Bacc()

    c_t = nc.declare_dram_parameter("c_t", [DH, NO], BF16, isOutput=False)
    a_t = nc.declare_dram_parameter("a_t", [DH, RPC], F32, isOutput=False)
    w2 = nc.declare_dram_parameter("w2", [DH, DOUT], BF16, isOutput=False)
    # out[s, p, (d, r, o)]: s = 4-row subgroup, p = dout within chunk,
    # free = d*1024 + r*256 + o. Host untransposes.
    out = nc.declare_dram_parameter("out", [NSUB, 128, 2048], BF16,
                                    isOutput=True)

    gelu = mybir.ActivationFunctionType.Gelu

    with TileContext(nc) as tc:
        with (
            tc.tile_pool(name="const", bufs=1) as cpool,
            tc.tile_pool(name="h", bufs=2) as hpool,
            tc.tile_pool(name="g", bufs=2) as gpool,
            tc.tile_pool(name="ps", bufs=2, space="PSUM") as pspool,
            tc.tile_pool(name="ostage", bufs=3) as opool,
        ):
            # Loads in gating order: the first adds need ct0+at0 only.
            ct, at, w2t = [], [], []
            for ch in range(2):
                t = cpool.tile([128, NO], BF16, tag=f"ct{ch}")
                ct.append(t)
                t = cpool.tile([128, RPC], F32, tag=f"at{ch}")
                at.append(t)
                t = cpool.tile([128, DOUT], BF16, tag=f"w2{ch}")
                w2t.append(t)
            nc.sync.dma_start(out=ct[0][:, :], in_=c_t[0:128, :])
            nc.sync.dma_start(out=at[0][:, :], in_=a_t[0:128, :])
            nc.scalar.dma_start(out=ct[1][:, :], in_=c_t[128:256, :])
            nc.scalar.dma_start(out=at[1][:, :], in_=a_t[128:256, :])
            for ch in range(2):
                nc.gpsimd.dma_start(out=w2t[ch][:, :],
                                    in_=w2[ch * 128:(ch + 1) * 128, :])

            # ACT warmup: pays the ~2.7us gelu table load during the ramp.
            # Memsets on the otherwise-idle gpsimd to keep DVE's queue clear.
            scratch = cpool.tile([128, 2], F32, tag="scratch")
            nc.gpsimd.memset(scratch[:, :], 0.0)
            nc.scalar.activation(scratch[:, :], scratch[:, :], gelu)

            # PE warmup: dummy matmuls flip the HAM clock-gate to 8/8
            # (2.4 GHz) before the first real matmul.
            dummy = cpool.tile([128, 128], BF16, tag="dummy")
            nc.gpsimd.memset(dummy[:, :], 0.0)
            ps_w = pspool.tile([128, 2048], F32, tag="ps")
            for i in range(12):
                nc.tensor.matmul(out=ps_w[:, 0:128], lhsT=dummy[:, :],
                                 rhs=dummy[:, :], start=True, stop=True)

            drain_idx = 0
            row0 = 0
            for g, nrows in enumerate(GROUP_ROWS):
                # h/g free layout: (ch, r, o) -> ch*nrows*256 + r*256 + o
                h_buf = hpool.tile([128, nrows * 512], BF16, tag="h")
                g_buf = gpool.tile([128, nrows * 512], BF16, tag="g")

                for ch in range(2):
                    for r in range(nrows):
                        row = row0 + r
                        s = (ch * nrows + r) * 256
                        nc.vector.tensor_scalar_add(
                            out=h_buf[:, s:s + 256],
                            in0=ct[ch][:, :],
                            scalar1=at[ch][:, row:row + 1],
                        )
                    nc.scalar.activation(
                        g_buf[:, ch * nrows * 256:(ch + 1) * nrows * 256],
                        h_buf[:, ch * nrows * 256:(ch + 1) * nrows * 256], gelu)

                # 4-row matmul subgroups, one [128,2048] PSUM tile each (2 in
                # flight), drains interleaved promptly so PE never waits long.
                # ps free layout: (d, rpair, o) -> d*1024 + j*512 + o'
                for s4 in range(nrows // 4):
                    sub = row0 // 4 + s4
                    ps = pspool.tile([128, 2048], F32, tag="ps")
                    for d in range(2):
                        for ch in range(2):
                            for j in range(2):
                                r0 = s4 * 4 + 2 * j
                                nc.tensor.matmul(
                                    out=ps[:, d * 1024 + j * 512:
                                           d * 1024 + (j + 1) * 512],
                                    lhsT=w2t[ch][:, d * 128:(d + 1) * 128],
                                    rhs=g_buf[:, (ch * nrows + r0) * 256:
                                              (ch * nrows + r0) * 256 + 512],
                                    start=(ch == 0), stop=(ch == 1),
                                )
                    ost = opool.tile([128, 2048], BF16, tag="ostage")
                    if (drain_idx * ACT_DRAINS) % NSUB < ACT_DRAINS:
                        nc.scalar.copy(ost[:, :], ps[:, :])
                    else:
                        nc.vector.tensor_copy(ost[:, :], ps[:, :])
                    drain_idx += 1
                    nc.sync.dma_start(out=out[sub], in_=ost[:, :])
                row0 += nrows

    nc.finalize()
    return nc


def _get_nc():
    global _nc_cache
    if _nc_cache is None:
        _nc_cache = build_nc()
    return _nc_cache


def make_in_maps(x, query, W_pre, b_pre, W_emb, b_emb, W1, b1, W2, b2):
    x = np.asarray(x, np.float32)
    query = np.asarray(query, np.float32)
    W_pre = np.asarray(W_pre, np.float32)
    b_pre = np.asarray(b_pre, np.float32)
    W_emb = np.asarray(W_emb, np.float32)
    b_emb = np.asarray(b_emb, np.float32)
    W1 = np.asarray(W1, np.float32)
    b1 = np.asarray(b1, np.float32)
    W2 = np.asarray(W2, np.float32)

    xp = x.reshape(B * NI, DIN) @ W_pre + b_pre
    A = xp @ W1[:DH] + b1                       # [B*NI, DH]
    c = query.reshape(B * NO, DQ) @ W_emb + b_emb
    C2 = c @ W1[DH:]                            # [B*NO, DH]
    A = A.reshape(B, NI, DH)
    C2 = C2.reshape(B, NO, DH)

    w2b = np.ascontiguousarray(W2.astype(ml_dtypes.bfloat16))
    in_maps = []
    for k in range(NCORES):
        b = k // 2
        hh = k % 2
        in_maps.append({
            "c_t": np.ascontiguousarray(C2[b].T.astype(ml_dtypes.bfloat16)),
            "a_t": np.ascontiguousarray(A[b, hh * 128:(hh + 1) * 128, :].T),
            "w2": w2b,
        })
    return in_maps


def run_on_device(in_maps, trace=False):
    nc = _get_nc()
    return run_bass_kernel_spmd(nc, in_maps, core_ids=list(range(NCORES)), trace=trace)


def assemble(results, b2):
    out = np.empty((B, NI, NO, DOUT), np.float32)
    for k in range(NCORES):
        b = k // 2
        hh = k % 2
        # dev out: [s, p, (d, r, o)] -> out[b, s*4+r, o, d*128+p]
        dev = results[k]["out"].reshape(NSUB, 128, 2, 4, 256)
        # axes (s, p, d, r, o) -> (s, r, o, d, p)
        dev = dev.transpose(0, 3, 4, 2, 1).reshape(RPC, NO, DOUT)
        out[b, hh * 128:(hh + 1) * 128] = dev.astype(np.float32)
    b2 = np.asarray(b2, np.float32)
    if np.any(b2):
        out += b2
    return out


def kernel(x, query, W_pre, b_pre, W_emb, b_emb, W1, b1, W2, b2):
    in_maps = make_in_maps(x, query, W_pre, b_pre, W_emb, b_emb, W1, b1, W2, b2)
    res = run_on_device(in_maps, trace=False)
    return assemble(res.results, b2)


# revision 16
# speedup vs baseline: 1.0531x; 1.0531x over previous
"""CondMlp Trainium2 kernel (v3).

Math (reference):
    xp = x @ W_pre + b_pre                 # [B, NI, DH]
    c  = query @ W_emb + b_emb             # [B, NO, DH]
    A  = xp @ W1[:DH] + b1                 # [B, NI, DH]   (host precompute, tiny)
    C2 = c @ W1[DH:]                       # [B, NO, DH]   (host precompute, tiny)
    h[b,i,o,:] = A[b,i,:] + C2[b,o,:]
    out[b,i,o,:] = gelu(h) @ W2 + b2       # [B, NI, NO, DOUT]

Sharding: 8 cores, core k handles batch b = k//2, NI-half h = k%2 (128 rows).

Design (measured constants from traces/microbench):
  * The kernel is ACT+DVE-bound: per core the gelu (58us, ACT-only), the
    broadcast adds (50us, DVE tensor_scalar at 2x_1P, 197ns/[128,256]) and
    the PSUM drains (1x port-bound; ACT 1.97us / DVE 2.29us per [128,2048])
    must share two engines -> ~89us/engine balanced.
  * bf16 output stores (halves DMA) assembled/untransposed on host.
  * W2-stationary matmuls, N=512: back-to-back same-lhsT MMs run at stream
    rate (216ns, LDWEIGHTS hidden by the PE background weight buffer).
  * Drains split ACT/DVE ~15/17 (the LP balance point).
  * 8-row first/last groups + per-subgroup 512KB stores shorten the
    pipeline head/tail; PE warmup matmuls flip the HAM clock-gate early.
  * GPSIMD stock tensor_scalar measured 3.9us per [128,256] add (20x DVE):
    useless for compute; its SWDGE ring only carries the w2 loads.
"""

import numpy as np
import ml_dtypes

import concourse.bass as bass
import concourse.bacc as bacc
import concourse.mybir as mybir
from concourse.tile import TileContext
from concourse.bass_utils import run_bass_kernel_spmd

B, NI, NO = 4, 256, 256
DIN, DQ, DH, DOUT = 256, 256, 256, 256
NCORES = 8
RPC = (B * NI) // NCORES    # rows per core = 128
NSUB = RPC // 4             # 32 matmul subgroups of 4 rows
GROUP_ROWS = [8, 8] + [16] * 7        # taper head; 16-row steady groups
F32 = mybir.dt.float32
BF16 = mybir.dt.bfloat16

ACT_DRAINS = 15             # of 32 drains, how many go to ACT (rest DVE)

_nc_cache = None


def build_nc():
    nc = bacc.Bacc()

    c_t = nc.declare_dram_parameter("c_t", [DH, NO], BF16, isOutput=False)
    a_t = nc.declare_dram_parameter("a_t", [DH, RPC], F32, isOutput=False)
    w2 = nc.declare_dram_parameter("w2", [DH, DOUT], BF16, isOutput=False)
    # out[s, p, (d, r, o)]: s = 4-row subgroup, p = dout within chunk,
    # free = d*1024 + r*256 + o. Host untransposes.
    out = nc.declare_dram_parameter("out", [NSUB, 128, 2048], BF16,
                                    isOutput=True)

    gelu = mybir.ActivationFunctionType.Gelu

    with TileContext(nc) as tc:
        with (
            tc.tile_pool(name="const", bufs=1) as cpool,
            tc.tile_pool(name="h", bufs=2) as hpool,
            tc.tile_pool(name="g", bufs=2) as gpool,
            tc.tile_pool(name="ps", bufs=2, space="PSUM") as pspool,
            tc.tile_pool(name="ostage", bufs=3) as opool,
        ):
            # Loads in gating order: the first adds need ct0+at0 only.
            ct, at, w2t = [], [], []
            for ch in range(2):
                t = cpool.tile([128, NO], BF16, tag=f"ct{ch}")
                ct.append(t)
                t = cpool.tile([128, RPC], F32, tag=f"at{ch}")
                at.append(t)
                t = cpool.tile([128, DOUT], BF16, tag=f"w2{ch}")
                w2t.append(t)
            nc.sync.dma_start(out=ct[0][:, :], in_=c_t[0:128, :])
            nc.sync.dma_start(out=at[0][:, :], in_=a_t[0:128, :])
            nc.scalar.dma_start(out=ct[1][:, :], in_=c_t[128:256, :])
            nc.scalar.dma_start(out=at[1][:, :], in_=a_t[128:256, :])
            for ch in range(2):
                nc.gpsimd.dma_start(out=w2t[ch][:, :],
                                    in_=w2[ch * 128:(ch + 1) * 128, :])

            # ACT warmup: pays the ~2.7us gelu table load during the ramp.
            # Memsets on the otherwise-idle gpsimd to keep DVE's queue clear.
            scratch = cpool.tile([128, 2], F32, tag="scratch")
            nc.gpsimd.memset(scratch[:, :], 0.0)
            nc.scalar.activation(scratch[:, :], scratch[:, :], gelu)

            # PE warmup: dummy matmuls flip the HAM clock-gate to 8/8
            # (2.4 GHz) before the first real matmul.
            dummy = cpool.tile([128, 128], BF16, tag="dummy")
            nc.gpsimd.memset(dummy[:, :], 0.0)
            ps_w = pspool.tile([128, 2048], F32, tag="ps")
            for i in range(12):
                nc.tensor.matmul(out=ps_w[:, 0:128], lhsT=dummy[:, :],
                                 rhs=dummy[:, :], start=True, stop=True)

            # --- software-pipelined main schedule -------------------------
            # Per 4-row subgroup "slot" of group g, the engine queues get:
            #   DVE: a chunk of group g+1's adds (ch0 chunks first), then
            #        this slot's drain if assigned here
            #   ACT: gelu(g+1, ch) as soon as its adds are queued, plus this
            #        slot's drain if assigned here (drains alternate engines
            #        per slot so a long gelu never head-of-line-blocks both)
            #   PE : this slot's 8 matmuls
            # h/g free layout: (ch, r, o) -> (ch*nrows + r)*256 + o

            def emit_adds(h_buf, nrows, row0, ch, rlist):
                for r in rlist:
                    s = (ch * nrows + r) * 256
                    nc.vector.tensor_scalar_add(
                        out=h_buf[:, s:s + 256],
                        in0=ct[ch][:, :],
                        scalar1=at[ch][:, row0 + r:row0 + r + 1],
                    )

            def emit_gelu(h_buf, g_buf, nrows, ch):
                nc.scalar.activation(
                    g_buf[:, ch * nrows * 256:(ch + 1) * nrows * 256],
                    h_buf[:, ch * nrows * 256:(ch + 1) * nrows * 256], gelu)

            ngroups = len(GROUP_ROWS)
            row_starts = []
            r0 = 0
            for nr in GROUP_ROWS:
                row_starts.append(r0)
                r0 += nr

            # Prologue: group 0 fully staged.
            h_cur = hpool.tile([128, GROUP_ROWS[0] * 512], BF16, tag="h")
            g_cur = gpool.tile([128, GROUP_ROWS[0] * 512], BF16, tag="g")
            for ch in range(2):
                emit_adds(h_cur, GROUP_ROWS[0], 0, ch, range(GROUP_ROWS[0]))
                emit_gelu(h_cur, g_cur, GROUP_ROWS[0], ch)

            drain_idx = 0
            for g in range(ngroups):
                nrows = GROUP_ROWS[g]
                row0 = row_starts[g]
                nslots = nrows // 4
                # Stage next group's adds/gelus across this group's slots.
                if g + 1 < ngroups:
                    nxt = GROUP_ROWS[g + 1]
                    h_nxt = hpool.tile([128, nxt * 512], BF16, tag="h")
                    g_nxt = gpool.tile([128, nxt * 512], BF16, tag="g")
                    tasks = ([(0, r) for r in range(nxt)] +
                             [(1, r) for r in range(nxt)])
                    per = -(-len(tasks) // nslots)
                    chunks = [tasks[i * per:(i + 1) * per]
                              for i in range(nslots)]
                else:
                    h_nxt = g_nxt = None
                    chunks = [[] for _ in range(nslots)]

                done_ch = [0, 0]
                for s4 in range(nslots):
                    sub = row0 // 4 + s4
                    for (ch, r) in chunks[s4]:
                        emit_adds(h_nxt, GROUP_ROWS[g + 1],
                                  row_starts[g + 1], ch, [r])
                        done_ch[ch] += 1
                        if done_ch[ch] == GROUP_ROWS[g + 1]:
                            emit_gelu(h_nxt, g_nxt, GROUP_ROWS[g + 1], ch)

                    ps = pspool.tile([128, 2048], F32, tag="ps")
                    # ps free layout: (d, rpair, o) -> d*1024 + j*512 + o'
                    for d in range(2):
                        for ch in range(2):
                            for j in range(2):
                                rr = s4 * 4 + 2 * j
                                nc.tensor.matmul(
                                    out=ps[:, d * 1024 + j * 512:
                                           d * 1024 + (j + 1) * 512],
                                    lhsT=w2t[ch][:, d * 128:(d + 1) * 128],
                                    rhs=g_cur[:, (ch * nrows + rr) * 256:
                                              (ch * nrows + rr) * 256 + 512],
                                    start=(ch == 0), stop=(ch == 1),
                                )
                    ost = opool.tile([128, 2048], BF16, tag="ostage")
                    if drain_idx % 2 == 0:
                        nc.scalar.copy(ost[:, :], ps[:, :])
                    else:
                        nc.vector.tensor_copy(ost[:, :], ps[:, :])
                    drain_idx += 1
                    nc.sync.dma_start(out=out[sub], in_=ost[:, :])

                h_cur, g_cur = h_nxt, g_nxt

    nc.finalize()
    return nc


def _get_nc():
    global _nc_cache
    if _nc_cache is None:
        _nc_cache = build_nc()
    return _nc_cache


def make_in_maps(x, query, W_pre, b_pre, W_emb, b_emb, W1, b1, W2, b2):
    x = np.asarray(x, np.float32)
    query = np.asarray(query, np.float32)
    W_pre = np.asarray(W_pre, np.float32)
    b_pre = np.asarray(b_pre, np.float32)
    W_emb = np.asarray(W_emb, np.float32)
    b_emb = np.asarray(b_emb, np.float32)
    W1 = np.asarray(W1, np.float32)
    b1 = np.asarray(b1, np.float32)
    W2 = np.asarray(W2, np.float32)

    xp = x.reshape(B * NI, DIN) @ W_pre + b_pre
    A = xp @ W1[:DH] + b1                       # [B*NI, DH]
    c = query.reshape(B * NO, DQ) @ W_emb + b_emb
    C2 = c @ W1[DH:]                            # [B*NO, DH]
    A = A.reshape(B, NI, DH)
    C2 = C2.reshape(B, NO, DH)

    w2b = np.ascontiguousarray(W2.astype(ml_dtypes.bfloat16))
    in_maps = []
    for k in range(NCORES):
        b = k // 2
        hh = k % 2
        in_maps.append({
            "c_t": np.ascontiguousarray(C2[b].T.astype(ml_dtypes.bfloat16)),
            "a_t": np.ascontiguousarray(A[b, hh * 128:(hh + 1) * 128, :].T),
            "w2": w2b,
        })
    return in_maps


def run_on_device(in_maps, trace=False):
    nc = _get_nc()
    return run_bass_kernel_spmd(nc, in_maps, core_ids=list(range(NCORES)), trace=trace)


def assemble(results, b2):
    out = np.empty((B, NI, NO, DOUT), np.float32)
    for k in range(NCORES):
        b = k // 2
        hh = k % 2
        # dev out: [s, p, (d, r, o)] -> out[b, s*4+r, o, d*128+p]
        dev = results[k]["out"].reshape(NSUB, 128, 2, 4, 256)
        # axes (s, p, d, r, o) -> (s, r, o, d, p)
        dev = dev.transpose(0, 3, 4, 2, 1).reshape(RPC, NO, DOUT)
        out[b, hh * 128:(hh + 1) * 128] = dev.astype(np.float32)
    b2 = np.asarray(b2, np.float32)
    if np.any(b2):
        out += b2
    return out


def kernel(x, query, W_pre, b_pre, W_emb, b_emb, W1, b1, W2, b2):
    in_maps = make_in_maps(x, query, W_pre, b_pre, W_emb, b_emb, W1, b1, W2, b2)
    res = run_on_device(in_maps, trace=False)
    return assemble(res.results, b2)


# revision 21
# speedup vs baseline: 1.0693x; 1.0154x over previous
"""CondMlp Trainium2 kernel (v3).

Math (reference):
    xp = x @ W_pre + b_pre                 # [B, NI, DH]
    c  = query @ W_emb + b_emb             # [B, NO, DH]
    A  = xp @ W1[:DH] + b1                 # [B, NI, DH]   (host precompute, tiny)
    C2 = c @ W1[DH:]                       # [B, NO, DH]   (host precompute, tiny)
    h[b,i,o,:] = A[b,i,:] + C2[b,o,:]
    out[b,i,o,:] = gelu(h) @ W2 + b2       # [B, NI, NO, DOUT]

Sharding: 8 cores, core k handles batch b = k//2, NI-half h = k%2 (128 rows).

Design (measured constants from traces/microbench):
  * The kernel is ACT+DVE-bound: per core the gelu (58us, ACT-only), the
    broadcast adds (50us, DVE tensor_scalar at 2x_1P, 197ns/[128,256]) and
    the PSUM drains (1x port-bound; ACT 1.97us / DVE 2.29us per [128,2048])
    must share two engines -> ~89us/engine balanced.
  * bf16 output stores (halves DMA) assembled/untransposed on host.
  * W2-stationary matmuls, N=512: back-to-back same-lhsT MMs run at stream
    rate (216ns, LDWEIGHTS hidden by the PE background weight buffer).
  * Drains split ACT/DVE ~15/17 (the LP balance point).
  * 8-row first/last groups + per-subgroup 512KB stores shorten the
    pipeline head/tail; PE warmup matmuls flip the HAM clock-gate early.
  * GPSIMD stock tensor_scalar measured 3.9us per [128,256] add (20x DVE):
    useless for compute; its SWDGE ring only carries the w2 loads.
"""

import numpy as np
import ml_dtypes

import concourse.bass as bass
import concourse.bacc as bacc
import concourse.mybir as mybir
from concourse.tile import TileContext
from concourse.bass_utils import run_bass_kernel_spmd

B, NI, NO = 4, 256, 256
DIN, DQ, DH, DOUT = 256, 256, 256, 256
NCORES = 8
RPC = (B * NI) // NCORES    # rows per core = 128
NSUB = RPC // 4             # 32 matmul subgroups of 4 rows
GROUP_ROWS = [8, 8] + [16] * 7        # taper head; 16-row steady groups
F32 = mybir.dt.float32
BF16 = mybir.dt.bfloat16

ACT_DRAINS = 15             # of 32 drains, how many go to ACT (rest DVE)

_nc_cache = None


def build_nc():
    nc = bacc.Bacc()

    c_t = nc.declare_dram_parameter("c_t", [DH, NO], BF16, isOutput=False)
    a_t = nc.declare_dram_parameter("a_t", [DH, RPC], F32, isOutput=False)
    # w2 as 4 contiguous [128,128] quadrants (ch, d): a contiguous stationary
    # is required for FWL (fast weight load) -> ~2x cheaper LDWEIGHTS.
    w2 = nc.declare_dram_parameter("w2", [4, 128, 128], BF16, isOutput=False)
    # out[s, p, (d, r, o)]: s = 4-row subgroup, p = dout within chunk,
    # free = d*1024 + r*256 + o. Host untransposes.
    out = nc.declare_dram_parameter("out", [NSUB, 128, 2048], BF16,
                                    isOutput=True)

    gelu = mybir.ActivationFunctionType.Gelu

    with TileContext(nc) as tc:
        with (
            tc.tile_pool(name="const", bufs=1) as cpool,
            tc.tile_pool(name="h", bufs=2) as hpool,
            tc.tile_pool(name="g", bufs=2) as gpool,
            tc.tile_pool(name="ps", bufs=2, space="PSUM") as pspool,
            tc.tile_pool(name="ostage", bufs=3) as opool,
        ):
            # Loads in gating order: the first adds need ct0+at0 only.
            ct, at, w2q = [], [], []
            for ch in range(2):
                t = cpool.tile([128, NO], BF16, tag=f"ct{ch}")
                ct.append(t)
                t = cpool.tile([128, RPC], F32, tag=f"at{ch}")
                at.append(t)
            for q in range(4):
                t = cpool.tile([128, 128], BF16, tag=f"w2q{q}")
                w2q.append(t)
            nc.sync.dma_start(out=ct[0][:, :], in_=c_t[0:128, :])
            nc.sync.dma_start(out=at[0][:, :], in_=a_t[0:128, :])
            nc.sync.dma_start(out=ct[1][:, :], in_=c_t[128:256, :])
            nc.sync.dma_start(out=at[1][:, :], in_=a_t[128:256, :])
            for q in range(4):
                nc.gpsimd.dma_start(out=w2q[q][:, :], in_=w2[q])

            # ACT warmup: pays the ~2.7us gelu table load during the ramp.
            # Memsets on the otherwise-idle gpsimd to keep DVE's queue clear.
            scratch = cpool.tile([128, 2], F32, tag="scratch")
            nc.gpsimd.memset(scratch[:, :], 0.0)
            nc.scalar.activation(scratch[:, :], scratch[:, :], gelu)

            # PE warmup: dummy matmuls flip the HAM clock-gate to 8/8
            # (2.4 GHz) before the first real matmul.
            dummy = cpool.tile([128, 128], BF16, tag="dummy")
            nc.gpsimd.memset(dummy[:, :], 0.0)
            ps_w = pspool.tile([128, 2048], F32, tag="ps")
            for i in range(12):
                nc.tensor.matmul(out=ps_w[:, 0:128], lhsT=dummy[:, :],
                                 rhs=dummy[:, :], start=True, stop=True)

            # --- software-pipelined main schedule -------------------------
            # Per 4-row subgroup "slot" of group g, the engine queues get:
            #   DVE: a chunk of group g+1's adds (ch0 chunks first), then
            #        this slot's drain if assigned here
            #   ACT: gelu(g+1, ch) as soon as its adds are queued, plus this
            #        slot's drain if assigned here (drains alternate engines
            #        per slot so a long gelu never head-of-line-blocks both)
            #   PE : this slot's 8 matmuls
            # h/g free layout: (ch, r, o) -> (ch*nrows + r)*256 + o

            def emit_adds(h_buf, nrows, row0, ch, rlist):
                for r in rlist:
                    s = (ch * nrows + r) * 256
                    nc.vector.tensor_scalar_add(
                        out=h_buf[:, s:s + 256],
                        in0=ct[ch][:, :],
                        scalar1=at[ch][:, row0 + r:row0 + r + 1],
                    )

            def emit_gelu(h_buf, g_buf, nrows, ch):
                nc.scalar.activation(
                    g_buf[:, ch * nrows * 256:(ch + 1) * nrows * 256],
                    h_buf[:, ch * nrows * 256:(ch + 1) * nrows * 256], gelu)

            ngroups = len(GROUP_ROWS)
            row_starts = []
            r0 = 0
            for nr in GROUP_ROWS:
                row_starts.append(r0)
                r0 += nr

            # Prologue: group 0 fully staged.
            h_cur = hpool.tile([128, GROUP_ROWS[0] * 512], BF16, tag="h")
            g_cur = gpool.tile([128, GROUP_ROWS[0] * 512], BF16, tag="g")
            for ch in range(2):
                emit_adds(h_cur, GROUP_ROWS[0], 0, ch, range(GROUP_ROWS[0]))
                emit_gelu(h_cur, g_cur, GROUP_ROWS[0], ch)

            drain_idx = 0
            for g in range(ngroups):
                nrows = GROUP_ROWS[g]
                row0 = row_starts[g]
                nslots = nrows // 4
                # Stage next group's adds/gelus across this group's slots.
                if g + 1 < ngroups:
                    nxt = GROUP_ROWS[g + 1]
                    h_nxt = hpool.tile([128, nxt * 512], BF16, tag="h")
                    g_nxt = gpool.tile([128, nxt * 512], BF16, tag="g")
                    tasks = ([(0, r) for r in range(nxt)] +
                             [(1, r) for r in range(nxt)])
                    per = -(-len(tasks) // nslots)
                    chunks = [tasks[i * per:(i + 1) * per]
                              for i in range(nslots)]
                else:
                    h_nxt = g_nxt = None
                    chunks = [[] for _ in range(nslots)]

                done_ch = [0, 0]
                for s4 in range(nslots):
                    sub = row0 // 4 + s4
                    for (ch, r) in chunks[s4]:
                        emit_adds(h_nxt, GROUP_ROWS[g + 1],
                                  row_starts[g + 1], ch, [r])
                        done_ch[ch] += 1
                        if done_ch[ch] == GROUP_ROWS[g + 1]:
                            emit_gelu(h_nxt, g_nxt, GROUP_ROWS[g + 1], ch)

                    ps = pspool.tile([128, 2048], F32, tag="ps")
                    # ps free layout: (d, rpair, o) -> d*1024 + j*512 + o'
                    for d in range(2):
                        for ch in range(2):
                            for j in range(2):
                                rr = s4 * 4 + 2 * j
                                nc.tensor.matmul(
                                    out=ps[:, d * 1024 + j * 512:
                                           d * 1024 + (j + 1) * 512],
                                    lhsT=w2q[ch * 2 + d][:, :],
                                    rhs=g_cur[:, (ch * nrows + rr) * 256:
                                              (ch * nrows + rr) * 256 + 512],
                                    start=(ch == 0), stop=(ch == 1),
                                )
                    ost = opool.tile([128, 2048], BF16, tag="ostage")
                    if drain_idx % 2 == 0:
                        nc.scalar.copy(ost[:, :], ps[:, :])
                    else:
                        nc.vector.tensor_copy(ost[:, :], ps[:, :])
                    drain_idx += 1
                    nc.sync.dma_start(out=out[sub], in_=ost[:, :])

                h_cur, g_cur = h_nxt, g_nxt

    nc.finalize()
    return nc


def _get_nc():
    global _nc_cache
    if _nc_cache is None:
        _nc_cache = build_nc()
    return _nc_cache


def make_in_maps(x, query, W_pre, b_pre, W_emb, b_emb, W1, b1, W2, b2):
    x = np.asarray(x, np.float32)
    query = np.asarray(query, np.float32)
    W_pre = np.asarray(W_pre, np.float32)
    b_pre = np.asarray(b_pre, np.float32)
    W_emb = np.asarray(W_emb, np.float32)
    b_emb = np.asarray(b_emb, np.float32)
    W1 = np.asarray(W1, np.float32)
    b1 = np.asarray(b1, np.float32)
    W2 = np.asarray(W2, np.float32)

    xp = x.reshape(B * NI, DIN) @ W_pre + b_pre
    A = xp @ W1[:DH] + b1                       # [B*NI, DH]
    c = query.reshape(B * NO, DQ) @ W_emb + b_emb
    C2 = c @ W1[DH:]                            # [B*NO, DH]
    A = A.reshape(B, NI, DH)
    C2 = C2.reshape(B, NO, DH)

    # w2 quadrants [ch*2+d] = W2[ch*128:(ch+1)*128, d*128:(d+1)*128]
    w2b = np.ascontiguousarray(
        W2.reshape(2, 128, 2, 128).transpose(0, 2, 1, 3).reshape(4, 128, 128)
        .astype(ml_dtypes.bfloat16))
    in_maps = []
    for k in range(NCORES):
        b = k // 2
        hh = k % 2
        in_maps.append({
            "c_t": np.ascontiguousarray(C2[b].T.astype(ml_dtypes.bfloat16)),
            "a_t": np.ascontiguousarray(A[b, hh * 128:(hh + 1) * 128, :].T),
            "w2": w2b,
        })
    return in_maps


def run_on_device(in_maps, trace=False):
    nc = _get_nc()
    return run_bass_kernel_spmd(nc, in_maps, core_ids=list(range(NCORES)), trace=trace)


def assemble(results, b2):
    out = np.empty((B, NI, NO, DOUT), np.float32)
    for k in range(NCORES):
        b = k // 2
        hh = k % 2
        # dev out: [s, p, (d, r, o)] -> out[b, s*4+r, o, d*128+p]
        dev = results[k]["out"].reshape(NSUB, 128, 2, 4, 256)
        # axes (s, p, d, r, o) -> (s, r, o, d, p)
        dev = dev.transpose(0, 3, 4, 2, 1).reshape(RPC, NO, DOUT)
        out[b, hh * 128:(hh + 1) * 128] = dev.astype(np.float32)
    b2 = np.asarray(b2, np.float32)
    if np.any(b2):
        out += b2
    return out


def kernel(x, query, W_pre, b_pre, W_emb, b_emb, W1, b1, W2, b2):
    in_maps = make_in_maps(x, query, W_pre, b_pre, W_emb, b_emb, W1, b1, W2, b2)
    res = run_on_device(in_maps, trace=False)
    return assemble(res.results, b2)


# revision 22
# speedup vs baseline: 1.1427x; 1.0687x over previous
"""CondMlp Trainium2 kernel.

Math (reference):
    xp = x @ W_pre + b_pre                 # [B, NI, DH]
    c  = query @ W_emb + b_emb             # [B, NO, DH]
    A  = xp @ W1[:DH] + b1                 # [B, NI, DH]   (host precompute, tiny)
    C2 = c @ W1[DH:]                       # [B, NO, DH]   (host precompute, tiny)
    h[b,i,o,:] = A[b,i,:] + C2[b,o,:]
    out[b,i,o,:] = gelu(h) @ W2 + b2       # [B, NI, NO, DOUT]

Sharding: 8 cores, core k handles batch b = k//2, NI-half h = k%2 (128 rows).

Design (constants measured from hardware traces/microbenchmarks):
  * The kernel is ACT+DVE-bound. Per core: gelu 59us (ACT-only, 1 elem/lane/
    cycle @1.2GHz), broadcast adds 50us (DVE tensor_scalar, 2x_1P cap with a
    tensor scalar operand, 196ns per [128,256]), PSUM drains 67us (1x
    port-bound fp32 reads; ACT 1.97us / DVE 2.29us per [128,2048]). Split
    across the two engines that's ~89us each; the matmuls (55us stream +
    ~25us LDWEIGHTS, partially hidden) keep PE at a similar level.
  * bf16 output stores (halves the 32MiB/core store traffic; ~0.2% rounding
    against a 2e-2 budget). Host untransposes + upcasts.
  * W2-stationary matmuls, N=512: out.T tiles = W2q.T @ g, so the gelu
    output feeds matmuls directly in its produced layout.
  * Drains alternate ACT/DVE per subgroup; stores are 1MiB (2 subgroups).
  * PE warmup matmuls flip the HAM clock-gate (1.2->2.4GHz) during the ramp;
    a scratch gelu pays the ~2.7us ACT table load early.
  * GPSIMD stock tensor_scalar measured 3.9us per [128,256] add (20x DVE):
    useless for compute; its ring only carries w2 loads + memsets.
"""

import numpy as np
import ml_dtypes

import concourse.bass as bass
import concourse.bacc as bacc
import concourse.mybir as mybir
from concourse.tile import TileContext
from concourse.bass_utils import run_bass_kernel_spmd

B, NI, NO = 4, 256, 256
DIN, DQ, DH, DOUT = 256, 256, 256, 256
NCORES = 8
RPC = (B * NI) // NCORES    # rows per core = 128
G16 = 16                    # rows per add/gelu group
NG = RPC // G16             # 8 groups
NSUB = RPC // 4             # 32 matmul subgroups of 4 rows
F32 = mybir.dt.float32
BF16 = mybir.dt.bfloat16

_nc_cache = None


def build_nc():
    nc = bacc.Bacc()

    c_t = nc.declare_dram_parameter("c_t", [DH, NO], BF16, isOutput=False)
    a_t = nc.declare_dram_parameter("a_t", [DH, RPC], F32, isOutput=False)
    # w2 quadrants [ch*2+d] = W2[ch*128:(ch+1)*128, d*128:(d+1)*128]
    w2 = nc.declare_dram_parameter("w2", [4, 128, 128], BF16, isOutput=False)
    # out[u, p, (s2, d, r, o)]: u = store unit (2 subgroups of 4 rows),
    # p = dout within chunk, free = s2*2048 + d*1024 + r*256 + o.
    out = nc.declare_dram_parameter("out", [NSUB // 2, 128, 4096], BF16,
                                    isOutput=True)

    gelu = mybir.ActivationFunctionType.Gelu

    with TileContext(nc) as tc:
        with (
            tc.tile_pool(name="const", bufs=1) as cpool,
            tc.tile_pool(name="h", bufs=2) as hpool,
            tc.tile_pool(name="g", bufs=2) as gpool,
            tc.tile_pool(name="ps", bufs=2, space="PSUM") as pspool,
            tc.tile_pool(name="ostage", bufs=3) as opool,
        ):
            ct, at, w2q = [], [], []
            for ch in range(2):
                t = cpool.tile([128, NO], BF16, tag=f"ct{ch}")
                ct.append(t)
                t = cpool.tile([128, RPC], F32, tag=f"at{ch}")
                at.append(t)
            for q in range(4):
                t = cpool.tile([128, 128], BF16, tag=f"w2q{q}")
                w2q.append(t)
            nc.sync.dma_start(out=ct[0][:, :], in_=c_t[0:128, :])
            nc.sync.dma_start(out=at[0][:, :], in_=a_t[0:128, :])
            nc.sync.dma_start(out=ct[1][:, :], in_=c_t[128:256, :])
            nc.sync.dma_start(out=at[1][:, :], in_=a_t[128:256, :])
            for q in range(4):
                nc.gpsimd.dma_start(out=w2q[q][:, :], in_=w2[q])

            # ACT warmup: pays the ~2.7us gelu table load during the ramp.
            scratch = cpool.tile([128, 2], F32, tag="scratch")
            nc.gpsimd.memset(scratch[:, :], 0.0)
            nc.scalar.activation(scratch[:, :], scratch[:, :], gelu)

            # PE warmup: dummy matmuls flip the HAM clock-gate to 8/8
            # (2.4 GHz) before the first real matmul.
            dummy = cpool.tile([128, 128], BF16, tag="dummy")
            nc.gpsimd.memset(dummy[:, :], 0.0)
            ps_w = pspool.tile([128, 2048], F32, tag="ps")
            for i in range(12):
                nc.tensor.matmul(out=ps_w[:, 0:128], lhsT=dummy[:, :],
                                 rhs=dummy[:, :], start=True, stop=True)

            drain_idx = 0
            for g in range(NG):
                # h/g free layout: (ch, r, o) -> ch*4096 + r*256 + o
                h_buf = hpool.tile([128, G16 * 512], BF16, tag="h")
                g_buf = gpool.tile([128, G16 * 512], BF16, tag="g")

                for ch in range(2):
                    for r in range(G16):
                        row = g * G16 + r
                        s = ch * 4096 + r * 256
                        nc.vector.tensor_scalar_add(
                            out=h_buf[:, s:s + 256],
                            in0=ct[ch][:, :],
                            scalar1=at[ch][:, row:row + 1],
                        )
                    nc.scalar.activation(
                        g_buf[:, ch * 4096:(ch + 1) * 4096],
                        h_buf[:, ch * 4096:(ch + 1) * 4096], gelu)

                # 4 subgroups of 4 rows; 2 subgroups share one 1 MiB store.
                for s2 in range(2):
                    ost = opool.tile([128, 4096], BF16, tag="ostage")
                    for s4i in range(2):
                        s4 = s2 * 2 + s4i
                        ps = pspool.tile([128, 2048], F32, tag="ps")
                        # ps free layout: (d, rpair, o) -> d*1024 + j*512 + o'
                        for d in range(2):
                            for ch in range(2):
                                for j in range(2):
                                    r0 = s4 * 4 + 2 * j
                                    nc.tensor.matmul(
                                        out=ps[:, d * 1024 + j * 512:
                                               d * 1024 + (j + 1) * 512],
                                        lhsT=w2q[ch * 2 + d][:, :],
                                        rhs=g_buf[:, ch * 4096 + r0 * 256:
                                                  ch * 4096 + r0 * 256 + 512],
                                        start=(ch == 0), stop=(ch == 1),
                                    )
                        dst = ost[:, s4i * 2048:(s4i + 1) * 2048]
                        if drain_idx % 2 == 1:
                            nc.scalar.copy(dst, ps[:, :])
                        else:
                            nc.vector.tensor_copy(dst, ps[:, :])
                        drain_idx += 1
                    u = g * 2 + s2
                    nc.sync.dma_start(out=out[u], in_=ost[:, :])

    nc.finalize()
    return nc


def _get_nc():
    global _nc_cache
    if _nc_cache is None:
        _nc_cache = build_nc()
    return _nc_cache


def make_in_maps(x, query, W_pre, b_pre, W_emb, b_emb, W1, b1, W2, b2):
    x = np.asarray(x, np.float32)
    query = np.asarray(query, np.float32)
    W_pre = np.asarray(W_pre, np.float32)
    b_pre = np.asarray(b_pre, np.float32)
    W_emb = np.asarray(W_emb, np.float32)
    b_emb = np.asarray(b_emb, np.float32)
    W1 = np.asarray(W1, np.float32)
    b1 = np.asarray(b1, np.float32)
    W2 = np.asarray(W2, np.float32)

    xp = x.reshape(B * NI, DIN) @ W_pre + b_pre
    A = xp @ W1[:DH] + b1                       # [B*NI, DH]
    c = query.reshape(B * NO, DQ) @ W_emb + b_emb
    C2 = c @ W1[DH:]                            # [B*NO, DH]
    A = A.reshape(B, NI, DH)
    C2 = C2.reshape(B, NO, DH)

    # w2 quadrants [ch*2+d] = W2[ch*128:(ch+1)*128, d*128:(d+1)*128]
    w2b = np.ascontiguousarray(
        W2.reshape(2, 128, 2, 128).transpose(0, 2, 1, 3).reshape(4, 128, 128)
        .astype(ml_dtypes.bfloat16))
    in_maps = []
    for k in range(NCORES):
        b = k // 2
        hh = k % 2
        in_maps.append({
            "c_t": np.ascontiguousarray(C2[b].T.astype(ml_dtypes.bfloat16)),
            "a_t": np.ascontiguousarray(A[b, hh * 128:(hh + 1) * 128, :].T),
            "w2": w2b,
        })
    return in_maps


def run_on_device(in_maps, trace=False):
    nc = _get_nc()
    return run_bass_kernel_spmd(nc, in_maps, core_ids=list(range(NCORES)), trace=trace)


def assemble(results, b2):
    out = np.empty((B, NI, NO, DOUT), np.float32)
    for k in range(NCORES):
        b = k // 2
        hh = k % 2
        # dev out: [u, p, (s2, d, r, o)] -> out[b, (u*2+s2)*4+r, o, d*128+p]
        dev = results[k]["out"].reshape(NSUB // 2, 128, 2, 2, 4, 256)
        # axes (u, p, s2, d, r, o) -> (u, s2, r, o, d, p)
        dev = dev.transpose(0, 2, 4, 5, 3, 1).reshape(RPC, NO, DOUT)
        out[b, hh * 128:(hh + 1) * 128] = dev.astype(np.float32)
    b2 = np.asarray(b2, np.float32)
    if np.any(b2):
        out += b2
    return out


def kernel(x, query, W_pre, b_pre, W_emb, b_emb, W1, b1, W2, b2):
    in_maps = make_in_maps(x, query, W_pre, b_pre, W_emb, b_emb, W1, b1, W2, b2)
    res = run_on_device(in_maps, trace=False)
    return assemble(res.results, b2)
